# revision 29
# baseline (speedup 1.0000x reference)
"""Ragged quantized-KV decode attention on 8 TRN2 NeuronCores.

Strategy
--------
Head-sharded: core c owns heads {2c, 2c+1} for ALL batches. Every batch's
ragged window [start_b, end_b) is identical across heads, so every core sees
the exact same chunk structure -> one SPMD program, perfectly load balanced.

Host side (not on the HW critical path): slice K/V to the ragged window,
fold k_scaler/sqrt(D) into K and v_scaler into V, pad each window to a
multiple of 128 positions, convert to bf16 and lay out as
[s_in_chunk=128, chunk, d] so every DMA reads long contiguous rows.

Device side (per core): for each 128-position chunk t of pair p=(b,h):
    qk[:, t]  = reduce_d(k_chunk * q_p)        (DVE mul + reduce, batched)
    qk[:, t] += mask[:, t]                     (DVE; -30000 on padding)
per pair:
    e = exp(qk_pair - M0), Lcol = sum_free(e)  (ACT, fused accum)
    Mcol = max_free(e)                         (DVE)
    psum[p, :] += e_col_t.T @ v_chunk_t        (PE, accumulated over chunks)
finale:
    l0 = colsum(Lcols) via ones-matmul, maxe = partition-allreduce(Mcols),
    out = psum * (1/l0)                        (DVE reciprocal + tensor_scalar)
Softmax uses a fixed shift M0 instead of the running max -- the normalized
output is shift-invariant; the true m and l are recovered from maxe/l0:
    m = M0 + log(maxe),   l = l0 / maxe.
"""

import math

import numpy as np
import ml_dtypes

B, H, S, D = 16, 16, 4096, 128
CH = 128  # positions per chunk (SBUF partition dim)
NCORES = 8
HPC = H // NCORES  # heads per core
NP = B * HPC  # pairs (b,h) per core
M0 = 12.0  # fixed softmax shift
NEG = -30000.0  # additive mask for padded positions
G = 32  # chunks per DMA slab

_PROGRAM_CACHE = {}


# ---------------------------------------------------------------- wait fixup
def _split_wide_waits(nc, maxw=1):
    """walrus in this container accepts at most 1 sem-wait per instruction;
    TileContext's exit drain aggregates many. Split into NoOp carriers."""
    import concourse.mybir as mybir

    for fn in nc.m.functions:
        for blk in fn.blocks:
            insts = list(blk.instructions)
            out = []
            changed = False
            for inst in insts:
                si = inst.sync_info
                ow = list(si.on_wait) if si is not None else []
                if len(ow) > maxw:
                    changed = True
                    k = 0
                    while len(ow) - k > maxw:
                        chunk = ow[k : k + maxw]
                        k += maxw
                        nop = mybir.InstNoOp(
                            name=nc.get_next_instruction_name(), ins=[], outs=[]
                        )
                        nop.engine = inst.engine
                        nop.sync_info = mybir.SyncInfo(on_wait=chunk, on_update=[])
                        nc.register_instruction(nop, overwrite=True)
                        out.append(nop)
                    si.on_wait = ow[k:]
                out.append(inst)
            if changed:
                blk.instructions[:] = out
    return nc


# ---------------------------------------------------------------- device code
def build_program(nch, g=G):
    """Build the SPMD Bass program. nch[b] = chunks for batch b's window.
    Identical for every core (each runs HPC heads of every batch)."""
    import concourse.bass as bass
    import concourse.mybir as mybir
    from concourse import tile

    np_pairs = len(nch) * HPC
    counts = []  # chunks per pair, pair order: (b, hh) b-major
    for c in nch:
        counts.extend([c] * HPC)
    t0s = np.concatenate([[0], np.cumsum(counts)]).astype(int)
    T = int(t0s[-1])

    bf16 = mybir.dt.bfloat16
    f32 = mybir.dt.float32

    nc = bass.Bass("TRN2", target_bir_lowering=False)
    kp_d = nc.declare_dram_parameter("kp", [CH, T, D], bf16, isOutput=False)
    vp_d = nc.declare_dram_parameter("vp", [CH, T, D], bf16, isOutput=False)
    qp_d = nc.declare_dram_parameter("qp", [D, np_pairs], bf16, isOutput=False)
    mask_d = nc.declare_dram_parameter("mask", [CH, T], bf16, isOutput=False)
    ident_d = nc.declare_dram_parameter("ident", [CH, CH], f32, isOutput=False)
    out_d = nc.declare_dram_parameter("out", [np_pairs, D], f32, isOutput=True)
    maxe_d = nc.declare_dram_parameter("maxe", [np_pairs, 1], f32, isOutput=True)
    l0_d = nc.declare_dram_parameter("l0", [1, np_pairs], f32, isOutput=True)

    n_slabs = (T + g - 1) // g

    # (pair, ta, tb) runs per slab; pair completion happens at its last run
    slab_runs = []
    for si in range(n_slabs):
        ga, gb = si * g, min((si + 1) * g, T)
        runs = []
        for p in range(np_pairs):
            a, b_ = max(t0s[p], ga), min(t0s[p + 1], gb)
            if a < b_:
                runs.append((p, a, b_))
        slab_runs.append(runs)

    with tile.TileContext(nc) as tc:
        with (
            tc.tile_pool(name="const", bufs=1) as constp,
            tc.tile_pool(name="kslab", bufs=4) as kpool,
            tc.tile_pool(name="vslab", bufs=6) as vpool,
            tc.tile_pool(name="prod", bufs=2) as prodp,
            tc.tile_pool(name="psum", bufs=2, space="PSUM") as psump,
        ):
            qp_sb = constp.tile([D, np_pairs], bf16)
            nc.scalar.dma_start(qp_sb[:], qp_d[:])
            mask_sb = constp.tile([CH, T], bf16)
            nc.scalar.dma_start(mask_sb[:], mask_d[:])
            ident_sb = constp.tile([CH, CH], f32)
            nc.scalar.dma_start(ident_sb[:], ident_d[:])

            qk_big = constp.tile([CH, T], f32)
            e_big = constp.tile([CH, T], bf16)
            lcols = constp.tile([CH, np_pairs], f32)
            mcols = constp.tile([CH, np_pairs], f32)
            ones = constp.tile([CH, 1], f32)
            nc.gpsimd.memset(ones[:], 1.0)
            negm0 = constp.tile([CH, 1], f32)
            nc.gpsimd.memset(negm0[:], -M0)
            l0_sb = constp.tile([1, np_pairs], f32)

            vtiles = [None] * n_slabs
            pending = []
            flush = []

            for si in range(n_slabs):
                ga, gb = si * g, min((si + 1) * g, T)
                w = gb - ga
                ksb = kpool.tile([CH, g, D], bf16, tag="k")
                vsb = vpool.tile([CH, g, D], bf16, tag="v")
                vtiles[si] = (vsb, ga)
                nc.sync.dma_start(ksb[:, :w, :], kp_d[:, ga:gb, :])
                nc.scalar.dma_start(vsb[:, :w, :], vp_d[:, ga:gb, :])

                qk_ps = psump.tile([CH, g], f32, tag="qk")
                for p, a, b_ in slab_runs[si]:
                    # qk columns via PE: kT_chunk (stationary, FWL) x q_col
                    for t in range(a, b_):
                        nc.tensor.matmul(
                            qk_ps[:, t - ga : t - ga + 1],
                            ksb[:, t - ga, :],
                            qp_sb[:, p : p + 1],
                            start=True,
                            stop=True,
                        )
                    nc.vector.tensor_add(
                        qk_big[:, a:b_],
                        qk_ps[:, a - ga : b_ - ga],
                        mask_sb[:, a:b_],
                    )

                    if b_ == t0s[p + 1]:  # pair complete
                        pa, pb = int(t0s[p]), int(t0s[p + 1])
                        nc.scalar.activation(
                            e_big[:, pa:pb],
                            qk_big[:, pa:pb],
                            mybir.ActivationFunctionType.Exp,
                            bias=negm0[:, :],
                            scale=1.0,
                            accum_out=lcols[:, p : p + 1],
                        )
                        nc.vector.reduce_max(
                            mcols[:, p : p + 1],
                            e_big[:, pa:pb],
                            axis=mybir.AxisListType.X,
                        )
                        # PV is deferred one slab so the next slab's qk
                        # matmuls hide the exp latency (PE is in-order).
                        pending.append((p, pa, pb))

                # P.V trains for pairs completed in the PREVIOUS slab
                for p, pa, pb in flush:
                    pv = psump.tile([1, D], f32, tag="pv")
                    for t in range(pa, pb):
                        vsb_t, vga = vtiles[t // g]
                        nc.tensor.matmul(
                            pv[:, :],
                            e_big[:, t : t + 1],
                            vsb_t[:, t - vga, :],
                            start=(t == pa),
                            stop=(t == pb - 1),
                        )
                    # l0 for this pair: cross-partition sum of lcols col
                    l0p = psump.tile([1, 1], f32, tag="l0")
                    nc.tensor.matmul(l0p[:, :], lcols[:, p : p + 1], ones[:, :])
                    rlp = prodp.tile([1, 1], f32, tag="rl")
                    nc.vector.reciprocal(rlp[:, :], l0p[:, :])
                    nc.vector.tensor_copy(l0_sb[:, p : p + 1], l0p[:, :])
                    # normalized output row -> DRAM
                    orow = prodp.tile([1, D], f32, tag="orow")
                    nc.scalar.mul(orow[:, :], pv[:, :], rlp[:, :])
                    nc.sync.dma_start(out_d[p : p + 1, :], orow[:, :])
                flush = pending
                pending = []

            # tail: PV trains for pairs completed in the last two slabs
            for p, pa, pb in flush + pending:
                pv = psump.tile([1, D], f32, tag="pv")
                for t in range(pa, pb):
                    vsb_t, vga = vtiles[t // g]
                    nc.tensor.matmul(
                        pv[:, :],
                        e_big[:, t : t + 1],
                        vsb_t[:, t - vga, :],
                        start=(t == pa),
                        stop=(t == pb - 1),
                    )
                l0p = psump.tile([1, 1], f32, tag="l0")
                nc.tensor.matmul(l0p[:, :], lcols[:, p : p + 1], ones[:, :])
                rlp = prodp.tile([1, 1], f32, tag="rl")
                nc.vector.reciprocal(rlp[:, :], l0p[:, :])
                nc.vector.tensor_copy(l0_sb[:, p : p + 1], l0p[:, :])
                orow = prodp.tile([1, D], f32, tag="orow")
                nc.scalar.mul(orow[:, :], pv[:, :], rlp[:, :])
                nc.sync.dma_start(out_d[p : p + 1, :], orow[:, :])

            # ---- finale: cross-partition max of mcols via PE transpose ----
            mtr = psump.tile([np_pairs, CH], f32, tag="mtr")
            nc.tensor.transpose(mtr[:, :], mcols[:, :], ident_sb[:, :])
            maxe_sb = constp.tile([np_pairs, 1], f32)
            nc.vector.reduce_max(maxe_sb[:, :], mtr[:, :], axis=mybir.AxisListType.X)
            nc.sync.dma_start(maxe_d[:], maxe_sb[:, :])
            nc.sync.dma_start(l0_d[:], l0_sb[:, :])

    _split_wide_waits(nc)
    return nc


# ---------------------------------------------------------------- host side
def _pack(q, k, v, k_scaler, v_scaler, start, end):
    """Slice/scale/pad/layout inputs. Returns per-core input maps + meta."""
    bf = ml_dtypes.bfloat16
    rsq = 1.0 / math.sqrt(D)

    idxs, nch = [], []
    for b in range(B):
        s0, e0 = int(start[b]), int(end[b])
        if s0 <= e0:
            idx = np.arange(s0, e0, dtype=np.int64)
        else:  # wrapped circular buffer
            idx = np.concatenate(
                [np.arange(s0, S, dtype=np.int64), np.arange(0, e0, dtype=np.int64)]
            )
        idxs.append(idx)
        nch.append(max(1, (len(idx) + CH - 1) // CH))

    TC = int(np.sum(nch))
    cum = np.concatenate([[0], np.cumsum(nch)]).astype(int)
    T = HPC * TC

    # global per-(b,h) packed KV in [s_in_chunk, h, chunk, d] layout
    gk = np.zeros((D, H, TC, CH), dtype=bf)
    gv = np.zeros((CH, H, TC, D), dtype=bf)
    gmask = np.full((CH, TC), NEG, dtype=bf)
    for b in range(B):
        idx, n = idxs[b], nch[b]
        L, Lp = len(idx), n * CH
        ksc = (k_scaler[b, idx] * rsq).astype(np.float32)
        vsc = v_scaler[b, idx].astype(np.float32)
        kw = k[b][:, idx, :] * ksc[None, :, None]  # [H, L, D]
        vw = v[b][:, idx, :] * vsc[None, :, None]
        if Lp > L:
            pad = ((0, 0), (0, Lp - L), (0, 0))
            kw = np.pad(kw, pad)
            vw = np.pad(vw, pad)
        # k transposed: [H, n, CH, D] -> [D, H, n, CH]  (d on partitions)
        gk[:, :, cum[b] : cum[b + 1], :] = (
            kw.reshape(H, n, CH, D).transpose(3, 0, 1, 2).astype(bf)
        )
        gv[:, :, cum[b] : cum[b + 1], :] = (
            vw.reshape(H, n, CH, D).transpose(2, 0, 1, 3).astype(bf)
        )
        mk = np.full(Lp, NEG, dtype=np.float32)
        mk[:L] = 0.0
        gmask[:, cum[b] : cum[b + 1]] = mk.reshape(n, CH).T

    in_maps = []
    for c in range(NCORES):
        kparts, vparts, mparts, qparts = [], [], [], []
        for b in range(B):
            for hh in range(HPC):
                h = HPC * c + hh
                kparts.append(gk[:, h, cum[b] : cum[b + 1], :])
                vparts.append(gv[:, h, cum[b] : cum[b + 1], :])
                mparts.append(gmask[:, cum[b] : cum[b + 1]])
                qparts.append(q[b, h][:, None].astype(bf))
        in_maps.append(
            {
                "kp": np.ascontiguousarray(np.concatenate(kparts, axis=1)),
                "vp": np.ascontiguousarray(np.concatenate(vparts, axis=1)),
                "mask": np.ascontiguousarray(np.concatenate(mparts, axis=1)),
                "qp": np.ascontiguousarray(np.concatenate(qparts, axis=1)),
                "ident": np.eye(CH, dtype=np.float32),
            }
        )
    return in_maps, tuple(nch), T


def kernel(q, k, v, k_scaler, v_scaler, start, end):
    q = np.asarray(q, dtype=np.float32)
    k = np.asarray(k, dtype=np.float32)
    v = np.asarray(v, dtype=np.float32)
    k_scaler = np.asarray(k_scaler, dtype=np.float32)
    v_scaler = np.asarray(v_scaler, dtype=np.float32)
    start = np.asarray(start)
    end = np.asarray(end)

    in_maps, nch, T = _pack(q, k, v, k_scaler, v_scaler, start, end)

    key = (nch, G)
    if key not in _PROGRAM_CACHE:
        _PROGRAM_CACHE[key] = build_program(list(nch))
    nc = _PROGRAM_CACHE[key]

    from concourse.bass_utils import run_bass_kernel_spmd

    res = run_bass_kernel_spmd(nc, in_maps, core_ids=list(range(NCORES)))
    global _LAST_RESULT
    _LAST_RESULT = res

    out = np.zeros((B, H, D), dtype=np.float32)
    m = np.zeros((B, H), dtype=np.float32)
    l = np.zeros((B, H), dtype=np.float32)
    for c in range(NCORES):
        r = res.results[c]
        o, me, l0 = r["out"], r["maxe"][:, 0], r["l0"][0]
        for b in range(B):
            for hh in range(HPC):
                p = HPC * b + hh
                h = HPC * c + hh
                out[b, h] = o[p]
                m[b, h] = M0 + np.log(me[p])
                l[b, h] = l0[p] / me[p]
    return out, (m, l)


# revision 30
# speedup vs baseline: 1.1478x; 1.1478x over previous
"""Ragged quantized-KV decode attention on 8 TRN2 NeuronCores.

Strategy
--------
Head-sharded: core c owns heads {2c, 2c+1} for ALL batches. Every batch's
ragged window [start_b, end_b) is identical across heads, so every core sees
the exact same chunk structure -> one SPMD program, perfectly load balanced.

Host side (not on the HW critical path): slice K/V to the ragged window,
fold k_scaler/sqrt(D) into K and v_scaler into V, pad each window to a
multiple of 128 positions, convert to bf16 and lay out as
[s_in_chunk=128, chunk, d] so every DMA reads long contiguous rows.

Device side (per core): for each 128-position chunk t of pair p=(b,h):
    qk[:, t]  = reduce_d(k_chunk * q_p)        (DVE mul + reduce, batched)
    qk[:, t] += mask[:, t]                     (DVE; -30000 on padding)
per pair:
    e = exp(qk_pair - M0), Lcol = sum_free(e)  (ACT, fused accum)
    Mcol = max_free(e)                         (DVE)
    psum[p, :] += e_col_t.T @ v_chunk_t        (PE, accumulated over chunks)
finale:
    l0 = colsum(Lcols) via ones-matmul, maxe = partition-allreduce(Mcols),
    out = psum * (1/l0)                        (DVE reciprocal + tensor_scalar)
Softmax uses a fixed shift M0 instead of the running max -- the normalized
output is shift-invariant; the true m and l are recovered from maxe/l0:
    m = M0 + log(maxe),   l = l0 / maxe.
"""

import math

import numpy as np
import ml_dtypes

B, H, S, D = 16, 16, 4096, 128
CH = 128  # positions per chunk (SBUF partition dim)
NCORES = 8
HPC = H // NCORES  # heads per core
NP = B * HPC  # pairs (b,h) per core
M0 = 12.0  # fixed softmax shift
NEG = -30000.0  # additive mask for padded positions
G = 32  # chunks per DMA slab

_PROGRAM_CACHE = {}


# ---------------------------------------------------------------- wait fixup
def _split_wide_waits(nc, maxw=1):
    """walrus in this container accepts at most 1 sem-wait per instruction;
    TileContext's exit drain aggregates many. Split into NoOp carriers."""
    import concourse.mybir as mybir

    for fn in nc.m.functions:
        for blk in fn.blocks:
            insts = list(blk.instructions)
            out = []
            changed = False
            for inst in insts:
                si = inst.sync_info
                ow = list(si.on_wait) if si is not None else []
                if len(ow) > maxw:
                    changed = True
                    k = 0
                    while len(ow) - k > maxw:
                        chunk = ow[k : k + maxw]
                        k += maxw
                        nop = mybir.InstNoOp(
                            name=nc.get_next_instruction_name(), ins=[], outs=[]
                        )
                        nop.engine = inst.engine
                        nop.sync_info = mybir.SyncInfo(on_wait=chunk, on_update=[])
                        nc.register_instruction(nop, overwrite=True)
                        out.append(nop)
                    si.on_wait = ow[k:]
                out.append(inst)
            if changed:
                blk.instructions[:] = out
    return nc


# ---------------------------------------------------------------- device code
def build_program(nch, g=G):
    """Build the SPMD Bass program. nch[b] = chunks for batch b's window.
    Identical for every core (each runs HPC heads of every batch)."""
    import concourse.bass as bass
    import concourse.mybir as mybir
    from concourse import tile

    np_pairs = len(nch) * HPC
    counts = []  # chunks per pair, pair order: (b, hh) b-major
    for c in nch:
        counts.extend([c] * HPC)
    t0s = np.concatenate([[0], np.cumsum(counts)]).astype(int)
    T = int(t0s[-1])

    bf16 = mybir.dt.bfloat16
    f32 = mybir.dt.float32

    nc = bass.Bass("TRN2", target_bir_lowering=False)
    kp_d = nc.declare_dram_parameter("kp", [CH, T, D], bf16, isOutput=False)
    vp_d = nc.declare_dram_parameter("vp", [CH, T, D], bf16, isOutput=False)
    qp_d = nc.declare_dram_parameter("qp", [D, np_pairs], bf16, isOutput=False)
    mask_d = nc.declare_dram_parameter("mask", [CH, T], bf16, isOutput=False)
    ident_d = nc.declare_dram_parameter("ident", [CH, CH], f32, isOutput=False)
    out_d = nc.declare_dram_parameter("out", [np_pairs, D], f32, isOutput=True)
    maxe_d = nc.declare_dram_parameter("maxe", [np_pairs, 1], f32, isOutput=True)
    l0_d = nc.declare_dram_parameter("l0", [1, np_pairs], f32, isOutput=True)

    n_slabs = (T + g - 1) // g

    # (pair, ta, tb) runs per slab; pair completion happens at its last run
    slab_runs = []
    for si in range(n_slabs):
        ga, gb = si * g, min((si + 1) * g, T)
        runs = []
        for p in range(np_pairs):
            a, b_ = max(t0s[p], ga), min(t0s[p + 1], gb)
            if a < b_:
                runs.append((p, a, b_))
        slab_runs.append(runs)

    with tile.TileContext(nc) as tc:
        with (
            tc.tile_pool(name="const", bufs=1) as constp,
            tc.tile_pool(name="kslab", bufs=4) as kpool,
            tc.tile_pool(name="vslab", bufs=4) as vpool,
            tc.tile_pool(name="prod", bufs=2) as prodp,
            tc.tile_pool(name="psum", bufs=2, space="PSUM") as psump,
        ):
            qp_sb = constp.tile([D, np_pairs], bf16)
            nc.scalar.dma_start(qp_sb[:], qp_d[:])
            mask_sb = constp.tile([CH, T], bf16)
            nc.scalar.dma_start(mask_sb[:], mask_d[:])
            ident_sb = constp.tile([CH, CH], f32)
            nc.scalar.dma_start(ident_sb[:], ident_d[:])

            qk_big = constp.tile([CH, T], f32)
            e_big = constp.tile([CH, T], bf16)
            lcols = constp.tile([CH, np_pairs], f32)
            mcols = constp.tile([CH, np_pairs], f32)
            ones = constp.tile([CH, 1], f32)
            nc.gpsimd.memset(ones[:], 1.0)
            negm0 = constp.tile([CH, 1], f32)
            nc.gpsimd.memset(negm0[:], -M0)
            l0_sb = constp.tile([1, np_pairs], f32)

            out_rows = constp.tile([1, np_pairs * D], f32)
            vtiles = [None] * n_slabs

            for si in range(n_slabs):
                ga, gb = si * g, min((si + 1) * g, T)
                w = gb - ga
                ksb = kpool.tile([CH, g, D], bf16, tag="k")
                vsb = vpool.tile([CH, g, D], bf16, tag="v")
                vtiles[si] = (vsb, ga)
                nc.sync.dma_start(ksb[:, :w, :], kp_d[:, ga:gb, :])
                nc.scalar.dma_start(vsb[:, :w, :], vp_d[:, ga:gb, :])

                qk_ps = psump.tile([CH, g], f32, tag="qk")
                for p, a, b_ in slab_runs[si]:
                    # qk columns via PE: kT_chunk (stationary, FWL) x q_col
                    for t in range(a, b_):
                        nc.tensor.matmul(
                            qk_ps[:, t - ga : t - ga + 1],
                            ksb[:, t - ga, :],
                            qp_sb[:, p : p + 1],
                            start=True,
                            stop=True,
                        )
                    nc.vector.tensor_add(
                        qk_big[:, a:b_],
                        qk_ps[:, a - ga : b_ - ga],
                        mask_sb[:, a:b_],
                    )

                    if b_ == t0s[p + 1]:  # pair complete
                        pa, pb = int(t0s[p]), int(t0s[p + 1])
                        nc.scalar.activation(
                            e_big[:, pa:pb],
                            qk_big[:, pa:pb],
                            mybir.ActivationFunctionType.Exp,
                            bias=negm0[:, :],
                            scale=1.0,
                            accum_out=lcols[:, p : p + 1],
                        )
                        nc.vector.reduce_max(
                            mcols[:, p : p + 1],
                            e_big[:, pa:pb],
                            axis=mybir.AxisListType.X,
                        )
                        # P.V accumulation, then stash the unnormalized
                        # row in partition-0 staging; normalization is one
                        # batched op at the end (keeps ACT free for exps).
                        pv = psump.tile([1, D], f32, tag="pv")
                        for t in range(pa, pb):
                            vsb_t, vga = vtiles[t // g]
                            nc.tensor.matmul(
                                pv[:, :],
                                e_big[:, t : t + 1],
                                vsb_t[:, t - vga, :],
                                start=(t == pa),
                                stop=(t == pb - 1),
                            )
                        nc.vector.tensor_copy(
                            out_rows[:, p * D : (p + 1) * D], pv[:, :]
                        )

            # ---- batched finale ----
            # l0 row: ones.T @ lcols -> [1, NP]; normalize all rows at once
            l0_ps = psump.tile([1, np_pairs], f32, tag="l0row")
            nc.tensor.matmul(l0_ps[:, :], ones[:, :], lcols[:, :])
            rl_row = constp.tile([1, np_pairs], f32)
            nc.vector.reciprocal(rl_row[:, :], l0_ps[:, :])
            nc.vector.tensor_copy(l0_sb[:, :], l0_ps[:, :])
            rl3 = rl_row[:, :].unsqueeze(2).broadcast_to([1, np_pairs, D])
            orows3 = out_rows[:, :].rearrange("a (p d) -> a p d", d=D)
            nc.vector.tensor_mul(orows3, orows3, rl3)
            nc.sync.dma_start(out_d[:, :], out_rows[:, :])

            # cross-partition max of mcols via PE transpose
            mtr = psump.tile([np_pairs, CH], f32, tag="mtr")
            nc.tensor.transpose(mtr[:, :], mcols[:, :], ident_sb[:, :])
            maxe_sb = constp.tile([np_pairs, 1], f32)
            nc.vector.reduce_max(maxe_sb[:, :], mtr[:, :], axis=mybir.AxisListType.X)
            nc.sync.dma_start(maxe_d[:], maxe_sb[:, :])
            nc.sync.dma_start(l0_d[:], l0_sb[:, :])

    _split_wide_waits(nc)
    return nc


# ---------------------------------------------------------------- host side
def _pack(q, k, v, k_scaler, v_scaler, start, end):
    """Slice/scale/pad/layout inputs. Returns per-core input maps + meta."""
    bf = ml_dtypes.bfloat16
    rsq = 1.0 / math.sqrt(D)

    idxs, nch = [], []
    for b in range(B):
        s0, e0 = int(start[b]), int(end[b])
        if s0 <= e0:
            idx = np.arange(s0, e0, dtype=np.int64)
        else:  # wrapped circular buffer
            idx = np.concatenate(
                [np.arange(s0, S, dtype=np.int64), np.arange(0, e0, dtype=np.int64)]
            )
        idxs.append(idx)
        nch.append(max(1, (len(idx) + CH - 1) // CH))

    TC = int(np.sum(nch))
    cum = np.concatenate([[0], np.cumsum(nch)]).astype(int)
    T = HPC * TC

    # global per-(b,h) packed KV in [s_in_chunk, h, chunk, d] layout
    gk = np.zeros((D, H, TC, CH), dtype=bf)
    gv = np.zeros((CH, H, TC, D), dtype=bf)
    gmask = np.full((CH, TC), NEG, dtype=bf)
    for b in range(B):
        idx, n = idxs[b], nch[b]
        L, Lp = len(idx), n * CH
        ksc = (k_scaler[b, idx] * rsq).astype(np.float32)
        vsc = v_scaler[b, idx].astype(np.float32)
        kw = k[b][:, idx, :] * ksc[None, :, None]  # [H, L, D]
        vw = v[b][:, idx, :] * vsc[None, :, None]
        if Lp > L:
            pad = ((0, 0), (0, Lp - L), (0, 0))
            kw = np.pad(kw, pad)
            vw = np.pad(vw, pad)
        # k transposed: [H, n, CH, D] -> [D, H, n, CH]  (d on partitions)
        gk[:, :, cum[b] : cum[b + 1], :] = (
            kw.reshape(H, n, CH, D).transpose(3, 0, 1, 2).astype(bf)
        )
        gv[:, :, cum[b] : cum[b + 1], :] = (
            vw.reshape(H, n, CH, D).transpose(2, 0, 1, 3).astype(bf)
        )
        mk = np.full(Lp, NEG, dtype=np.float32)
        mk[:L] = 0.0
        gmask[:, cum[b] : cum[b + 1]] = mk.reshape(n, CH).T

    in_maps = []
    for c in range(NCORES):
        kparts, vparts, mparts, qparts = [], [], [], []
        for b in range(B):
            for hh in range(HPC):
                h = HPC * c + hh
                kparts.append(gk[:, h, cum[b] : cum[b + 1], :])
                vparts.append(gv[:, h, cum[b] : cum[b + 1], :])
                mparts.append(gmask[:, cum[b] : cum[b + 1]])
                qparts.append(q[b, h][:, None].astype(bf))
        in_maps.append(
            {
                "kp": np.ascontiguousarray(np.concatenate(kparts, axis=1)),
                "vp": np.ascontiguousarray(np.concatenate(vparts, axis=1)),
                "mask": np.ascontiguousarray(np.concatenate(mparts, axis=1)),
                "qp": np.ascontiguousarray(np.concatenate(qparts, axis=1)),
                "ident": np.eye(CH, dtype=np.float32),
            }
        )
    return in_maps, tuple(nch), T


def kernel(q, k, v, k_scaler, v_scaler, start, end):
    q = np.asarray(q, dtype=np.float32)
    k = np.asarray(k, dtype=np.float32)
    v = np.asarray(v, dtype=np.float32)
    k_scaler = np.asarray(k_scaler, dtype=np.float32)
    v_scaler = np.asarray(v_scaler, dtype=np.float32)
    start = np.asarray(start)
    end = np.asarray(end)

    in_maps, nch, T = _pack(q, k, v, k_scaler, v_scaler, start, end)

    key = (nch, G)
    if key not in _PROGRAM_CACHE:
        _PROGRAM_CACHE[key] = build_program(list(nch))
    nc = _PROGRAM_CACHE[key]

    from concourse.bass_utils import run_bass_kernel_spmd

    res = run_bass_kernel_spmd(nc, in_maps, core_ids=list(range(NCORES)))
    global _LAST_RESULT
    _LAST_RESULT = res

    out = np.zeros((B, H, D), dtype=np.float32)
    m = np.zeros((B, H), dtype=np.float32)
    l = np.zeros((B, H), dtype=np.float32)
    for c in range(NCORES):
        r = res.results[c]
        o, me, l0 = r["out"], r["maxe"][:, 0], r["l0"][0]
        for b in range(B):
            for hh in range(HPC):
                p = HPC * b + hh
                h = HPC * c + hh
                out[b, h] = o[p]
                m[b, h] = M0 + np.log(me[p])
                l[b, h] = l0[p] / me[p]
    return out, (m, l)


# revision 31
# speedup vs baseline: 1.2431x; 1.0831x over previous
"""Ragged quantized-KV decode attention on 8 TRN2 NeuronCores.

Strategy
--------
Head-sharded: core c owns heads {2c, 2c+1} for ALL batches. Every batch's
ragged window [start_b, end_b) is identical across heads, so every core sees
the exact same chunk structure -> one SPMD program, perfectly load balanced.

Host side (not on the HW critical path): slice K/V to the ragged window,
fold k_scaler/sqrt(D) into K and v_scaler into V, pad each window to a
multiple of 128 positions, convert to bf16 and lay out as
[s_in_chunk=128, chunk, d] so every DMA reads long contiguous rows.

Device side (per core): for each 128-position chunk t of pair p=(b,h):
    qk[:, t]  = reduce_d(k_chunk * q_p)        (DVE mul + reduce, batched)
    qk[:, t] += mask[:, t]                     (DVE; -30000 on padding)
per pair:
    e = exp(qk_pair - M0), Lcol = sum_free(e)  (ACT, fused accum)
    Mcol = max_free(e)                         (DVE)
    psum[p, :] += e_col_t.T @ v_chunk_t        (PE, accumulated over chunks)
finale:
    l0 = colsum(Lcols) via ones-matmul, maxe = partition-allreduce(Mcols),
    out = psum * (1/l0)                        (DVE reciprocal + tensor_scalar)
Softmax uses a fixed shift M0 instead of the running max -- the normalized
output is shift-invariant; the true m and l are recovered from maxe/l0:
    m = M0 + log(maxe),   l = l0 / maxe.
"""

import math

import numpy as np
import ml_dtypes

B, H, S, D = 16, 16, 4096, 128
CH = 128  # positions per chunk (SBUF partition dim)
NCORES = 8
HPC = H // NCORES  # heads per core
NP = B * HPC  # pairs (b,h) per core
M0 = 12.0  # fixed softmax shift
NEG = -30000.0  # additive mask for padded positions
G = 32  # chunks per DMA slab

_PROGRAM_CACHE = {}


# ---------------------------------------------------------------- wait fixup
def _split_wide_waits(nc, maxw=1):
    """walrus in this container accepts at most 1 sem-wait per instruction;
    TileContext's exit drain aggregates many. Split into NoOp carriers."""
    import concourse.mybir as mybir

    for fn in nc.m.functions:
        for blk in fn.blocks:
            insts = list(blk.instructions)
            out = []
            changed = False
            for inst in insts:
                si = inst.sync_info
                ow = list(si.on_wait) if si is not None else []
                if len(ow) > maxw:
                    changed = True
                    k = 0
                    while len(ow) - k > maxw:
                        chunk = ow[k : k + maxw]
                        k += maxw
                        nop = mybir.InstNoOp(
                            name=nc.get_next_instruction_name(), ins=[], outs=[]
                        )
                        nop.engine = inst.engine
                        nop.sync_info = mybir.SyncInfo(on_wait=chunk, on_update=[])
                        nc.register_instruction(nop, overwrite=True)
                        out.append(nop)
                    si.on_wait = ow[k:]
                out.append(inst)
            if changed:
                blk.instructions[:] = out
    return nc


# ---------------------------------------------------------------- device code
def build_program(nch, g=G):
    """Build the SPMD Bass program. nch[b] = chunks for batch b's window.
    Identical for every core (each runs HPC heads of every batch)."""
    import concourse.bass as bass
    import concourse.mybir as mybir
    from concourse import tile

    np_pairs = len(nch) * HPC
    counts = []  # chunks per pair, pair order: (b, hh) b-major
    for c in nch:
        counts.extend([c] * HPC)
    t0s = np.concatenate([[0], np.cumsum(counts)]).astype(int)
    T = int(t0s[-1])

    bf16 = mybir.dt.bfloat16
    f32 = mybir.dt.float32

    nc = bass.Bass("TRN2", target_bir_lowering=False)
    kp_d = nc.declare_dram_parameter("kp", [CH, T, D], bf16, isOutput=False)
    vp_d = nc.declare_dram_parameter("vp", [CH, T, D], bf16, isOutput=False)
    qp_d = nc.declare_dram_parameter("qp", [D, np_pairs], bf16, isOutput=False)
    mask_d = nc.declare_dram_parameter("mask", [CH, T], bf16, isOutput=False)
    ident_d = nc.declare_dram_parameter("ident", [CH, CH], f32, isOutput=False)
    out_d = nc.declare_dram_parameter("out", [np_pairs, D], f32, isOutput=True)
    maxe_d = nc.declare_dram_parameter("maxe", [np_pairs, 1], f32, isOutput=True)
    l0_d = nc.declare_dram_parameter("l0", [1, np_pairs], f32, isOutput=True)

    n_slabs = (T + g - 1) // g

    # (pair, ta, tb) runs per slab; pair completion happens at its last run
    slab_runs = []
    for si in range(n_slabs):
        ga, gb = si * g, min((si + 1) * g, T)
        runs = []
        for p in range(np_pairs):
            a, b_ = max(t0s[p], ga), min(t0s[p + 1], gb)
            if a < b_:
                runs.append((p, a, b_))
        slab_runs.append(runs)

    with tile.TileContext(nc) as tc:
        with (
            tc.tile_pool(name="const", bufs=1) as constp,
            tc.tile_pool(name="kslab", bufs=5) as kpool,
            tc.tile_pool(name="vslab", bufs=5) as vpool,
            tc.tile_pool(name="prod", bufs=2) as prodp,
            tc.tile_pool(name="psum", bufs=2, space="PSUM") as psump,
        ):
            qp_sb = constp.tile([D, np_pairs], bf16)
            nc.scalar.dma_start(qp_sb[:], qp_d[:])
            mask_sb = constp.tile([CH, T], bf16)
            nc.scalar.dma_start(mask_sb[:], mask_d[:])
            ident_sb = constp.tile([CH, CH], f32)
            nc.scalar.dma_start(ident_sb[:], ident_d[:])

            qk_big = constp.tile([CH, T], f32)
            e_big = constp.tile([CH, T], bf16)
            lcols = constp.tile([CH, np_pairs], f32)
            mcols = constp.tile([CH, np_pairs], f32)
            ones = constp.tile([CH, 1], f32)
            nc.gpsimd.memset(ones[:], 1.0)
            negm0 = constp.tile([CH, 1], f32)
            nc.gpsimd.memset(negm0[:], -M0)
            l0_sb = constp.tile([1, np_pairs], f32)

            out_rows = constp.tile([1, np_pairs * D], f32)
            rl_row = constp.tile([1, np_pairs], f32)
            GN = 8  # pairs per normalize group (np_pairs % GN == 0)
            vtiles = [None] * n_slabs

            for si in range(n_slabs):
                ga, gb = si * g, min((si + 1) * g, T)
                w = gb - ga
                ksb = kpool.tile([CH, g, D], bf16, tag="k")
                vsb = vpool.tile([CH, g, D], bf16, tag="v")
                vtiles[si] = (vsb, ga)
                nc.sync.dma_start(ksb[:, :w, :], kp_d[:, ga:gb, :])
                nc.scalar.dma_start(vsb[:, :w, :], vp_d[:, ga:gb, :])

                qk_ps = psump.tile([CH, g], f32, tag="qk")
                for p, a, b_ in slab_runs[si]:
                    # qk columns via PE: kT_chunk (stationary, FWL) x q_col
                    for t in range(a, b_):
                        nc.tensor.matmul(
                            qk_ps[:, t - ga : t - ga + 1],
                            ksb[:, t - ga, :],
                            qp_sb[:, p : p + 1],
                            start=True,
                            stop=True,
                        )
                    nc.vector.tensor_add(
                        qk_big[:, a:b_],
                        qk_ps[:, a - ga : b_ - ga],
                        mask_sb[:, a:b_],
                    )

                    if b_ == t0s[p + 1]:  # pair complete
                        pa, pb = int(t0s[p]), int(t0s[p + 1])
                        nc.scalar.activation(
                            e_big[:, pa:pb],
                            qk_big[:, pa:pb],
                            mybir.ActivationFunctionType.Exp,
                            bias=negm0[:, :],
                            scale=1.0,
                            accum_out=lcols[:, p : p + 1],
                        )
                        nc.vector.reduce_max(
                            mcols[:, p : p + 1],
                            e_big[:, pa:pb],
                            axis=mybir.AxisListType.X,
                        )
                        # P.V accumulation, then stash the unnormalized
                        # row in partition-0 staging; normalization is one
                        # batched op at the end (keeps ACT free for exps).
                        pv = psump.tile([1, D], f32, tag="pv")
                        for t in range(pa, pb):
                            vsb_t, vga = vtiles[t // g]
                            nc.tensor.matmul(
                                pv[:, :],
                                e_big[:, t : t + 1],
                                vsb_t[:, t - vga, :],
                                start=(t == pa),
                                stop=(t == pb - 1),
                            )
                        nc.vector.tensor_copy(
                            out_rows[:, p * D : (p + 1) * D], pv[:, :]
                        )
                        if p % GN == GN - 1:  # normalize + ship this group
                            ga_, gb_ = p - GN + 1, p + 1
                            l0_ps = psump.tile([1, GN], f32, tag="l0row")
                            nc.tensor.matmul(
                                l0_ps[:, :], ones[:, :], lcols[:, ga_:gb_]
                            )
                            nc.vector.reciprocal(
                                rl_row[:, ga_:gb_], l0_ps[:, :]
                            )
                            nc.vector.tensor_copy(
                                l0_sb[:, ga_:gb_], l0_ps[:, :]
                            )
                            rl3 = (
                                rl_row[:, ga_:gb_]
                                .unsqueeze(2)
                                .broadcast_to([1, GN, D])
                            )
                            seg = out_rows[:, ga_ * D : gb_ * D].rearrange(
                                "a (p d) -> a p d", d=D
                            )
                            nc.vector.tensor_mul(seg, seg, rl3)
                            nc.sync.dma_start(
                                out_d[ga_:gb_, :],
                                out_rows[:, ga_ * D : gb_ * D],
                            )

            # ---- finale ----
            # cross-partition max of mcols via PE transpose
            mtr = psump.tile([np_pairs, CH], f32, tag="mtr")
            nc.tensor.transpose(mtr[:, :], mcols[:, :], ident_sb[:, :])
            maxe_sb = constp.tile([np_pairs, 1], f32)
            nc.vector.reduce_max(maxe_sb[:, :], mtr[:, :], axis=mybir.AxisListType.X)
            nc.sync.dma_start(maxe_d[:], maxe_sb[:, :])
            nc.sync.dma_start(l0_d[:], l0_sb[:, :])

    _split_wide_waits(nc)
    return nc


# ---------------------------------------------------------------- host side
def _pack(q, k, v, k_scaler, v_scaler, start, end):
    """Slice/scale/pad/layout inputs. Returns per-core input maps + meta."""
    bf = ml_dtypes.bfloat16
    rsq = 1.0 / math.sqrt(D)

    idxs, nch = [], []
    for b in range(B):
        s0, e0 = int(start[b]), int(end[b])
        if s0 <= e0:
            idx = np.arange(s0, e0, dtype=np.int64)
        else:  # wrapped circular buffer
            idx = np.concatenate(
                [np.arange(s0, S, dtype=np.int64), np.arange(0, e0, dtype=np.int64)]
            )
        idxs.append(idx)
        nch.append(max(1, (len(idx) + CH - 1) // CH))

    TC = int(np.sum(nch))
    cum = np.concatenate([[0], np.cumsum(nch)]).astype(int)
    T = HPC * TC

    # global per-(b,h) packed KV in [s_in_chunk, h, chunk, d] layout
    gk = np.zeros((D, H, TC, CH), dtype=bf)
    gv = np.zeros((CH, H, TC, D), dtype=bf)
    gmask = np.full((CH, TC), NEG, dtype=bf)
    for b in range(B):
        idx, n = idxs[b], nch[b]
        L, Lp = len(idx), n * CH
        ksc = (k_scaler[b, idx] * rsq).astype(np.float32)
        vsc = v_scaler[b, idx].astype(np.float32)
        kw = k[b][:, idx, :] * ksc[None, :, None]  # [H, L, D]
        vw = v[b][:, idx, :] * vsc[None, :, None]
        if Lp > L:
            pad = ((0, 0), (0, Lp - L), (0, 0))
            kw = np.pad(kw, pad)
            vw = np.pad(vw, pad)
        # k transposed: [H, n, CH, D] -> [D, H, n, CH]  (d on partitions)
        gk[:, :, cum[b] : cum[b + 1], :] = (
            kw.reshape(H, n, CH, D).transpose(3, 0, 1, 2).astype(bf)
        )
        gv[:, :, cum[b] : cum[b + 1], :] = (
            vw.reshape(H, n, CH, D).transpose(2, 0, 1, 3).astype(bf)
        )
        mk = np.full(Lp, NEG, dtype=np.float32)
        mk[:L] = 0.0
        gmask[:, cum[b] : cum[b + 1]] = mk.reshape(n, CH).T

    in_maps = []
    for c in range(NCORES):
        kparts, vparts, mparts, qparts = [], [], [], []
        for b in range(B):
            for hh in range(HPC):
                h = HPC * c + hh
                kparts.append(gk[:, h, cum[b] : cum[b + 1], :])
                vparts.append(gv[:, h, cum[b] : cum[b + 1], :])
                mparts.append(gmask[:, cum[b] : cum[b + 1]])
                qparts.append(q[b, h][:, None].astype(bf))
        in_maps.append(
            {
                "kp": np.ascontiguousarray(np.concatenate(kparts, axis=1)),
                "vp": np.ascontiguousarray(np.concatenate(vparts, axis=1)),
                "mask": np.ascontiguousarray(np.concatenate(mparts, axis=1)),
                "qp": np.ascontiguousarray(np.concatenate(qparts, axis=1)),
                "ident": np.eye(CH, dtype=np.float32),
            }
        )
    return in_maps, tuple(nch), T


def kernel(q, k, v, k_scaler, v_scaler, start, end):
    q = np.asarray(q, dtype=np.float32)
    k = np.asarray(k, dtype=np.float32)
    v = np.asarray(v, dtype=np.float32)
    k_scaler = np.asarray(k_scaler, dtype=np.float32)
    v_scaler = np.asarray(v_scaler, dtype=np.float32)
    start = np.asarray(start)
    end = np.asarray(end)

    in_maps, nch, T = _pack(q, k, v, k_scaler, v_scaler, start, end)

    key = (nch, G)
    if key not in _PROGRAM_CACHE:
        _PROGRAM_CACHE[key] = build_program(list(nch))
    nc = _PROGRAM_CACHE[key]

    from concourse.bass_utils import run_bass_kernel_spmd

    res = run_bass_kernel_spmd(nc, in_maps, core_ids=list(range(NCORES)))
    global _LAST_RESULT
    _LAST_RESULT = res

    out = np.zeros((B, H, D), dtype=np.float32)
    m = np.zeros((B, H), dtype=np.float32)
    l = np.zeros((B, H), dtype=np.float32)
    for c in range(NCORES):
        r = res.results[c]
        o, me, l0 = r["out"], r["maxe"][:, 0], r["l0"][0]
        for b in range(B):
            for hh in range(HPC):
                p = HPC * b + hh
                h = HPC * c + hh
                out[b, h] = o[p]
                m[b, h] = M0 + np.log(me[p])
                l[b, h] = l0[p] / me[p]
    return out, (m, l)
